# revision 16
# baseline (speedup 1.0000x reference)
"""Distributed attention kernel for 8 TRN2 NeuronCores.

Problem: B=2, S=2048, D=1024, H=16 heads (hd=64), no causal mask, no
scaling.  out = softmax((x@Wq) (x@Wk)^T) (x@Wv) @ Wp + biases.

Sharding: DP=2 over batch x TP=4 over heads.  Core c handles batch c//4
and heads 4*(c%4) .. 4*(c%4)+3, computes its 4 heads' attention plus the
partial c_proj, then a chunked bf16 ReduceScatter(add) over its 4-core
group yields each core's 512-row slice of the final output.

Design notes:
- Scores for a HEAD PAIR are emitted as row-tiled matmul pairs: head A
  occupies PE row strips 0-1 (K rows 0-63), head B strips 2-3 (rows
  64-127); tile_position is inferred from the lhsT/rhs base partitions.
  Adjacent instructions with disjoint row groups execute concurrently in
  the PE array, nearly halving scores time (the K=64 contraction only
  fills half the array otherwise).
- Stages are head-pairs (16 exp instructions each on the ACT engine).
  All other PE work (k/v/q projections for later stages, c_proj of the
  previous chunk, PV matmuls) drains through a cycle-budgeted FIFO work
  queue BEFORE each score slot, so a dependency-stalled scores matmul
  never blocks ready work in the in-order PE queue.
- PV matmuls are rolling-pushed right after their exp instructions in
  4-matmul units (fewer PSUM bank re-entries) and drain ~1 stage later.
- Only the first x column-chunk is DMAed up front so the first QKV
  group starts after ~2.5MB instead of 5.6MB of DMA.
- v-bias and c_proj bias are applied host-side (probs sum to 1, so the
  v-bias shifts attention output by b_v exactly); this removes 48 bias
  matmuls.
- The collective stream runs ReduceScatter pieces serially; the last
  chunk uses a 384+128 row split so the final (exposed) piece is small.

Softmax skips the max-subtraction (scores are O(+-20), exp is safe in
f32): probs = exp(s) / sum exp(s); the denominator comes free as the
65th row of the PV matmul via an appended ones-column on V.

Measured on the 8-core axon trn2 terminal: ~348us (baseline 389us),
rel err 9.56e-3.
"""

import sys
from collections import deque

if "/opt/trn_rl_repo" not in sys.path:
    sys.path.insert(0, "/opt/trn_rl_repo")

import numpy as np
import ml_dtypes

import concourse.bass as bass
import concourse.mybir as mybir
from concourse import bacc
from concourse.tile import TileContext
from concourse.bass_utils import run_bass_kernel_spmd

BF16 = mybir.dt.bfloat16
F32 = mybir.dt.float32

B, S, D = 2, 2048, 1024
H = 16
HD = 64
TP = 4  # tensor-parallel group size (cores per batch)
HPC = H // TP  # heads per core = 4
QC = HPC * HD  # q (or k or v) columns per core = 256
SQB = 512  # sq chunk (free dim of scores/pv matmuls)
NJ = S // SQB  # 4 chunks
NT = S // 128  # 16 sk tiles
NK = D // 128  # 8 contraction tiles for the projections
SO = S // TP  # 512 output rows per core

SLOT_BUDGET = 4200  # queue cycles emitted per score slot
CREDIT_CAP = 8400

# Reduce-scatter pieces (row0, nrows).  Each piece reduces
# partial[row0:row0+nrows] into out_ext[row0//TP:(row0+nrows)//TP]; core
# with group rank g receives reduced rows [row0+g*nrows/TP, +nrows/TP).
# The collective stream executes pieces serially and each piece costs
# ~9.5us fixed + ~18ns/row, so one 512-row piece per chunk is optimal.
RS_PIECES = [(j * 512, 512) for j in range(NJ)]

_CACHE = {}


def build():
    nc = bacc.Bacc(num_devices=8)

    xT_ext = nc.declare_dram_parameter("xT", [D, S], BF16, isOutput=False)
    wqkv_ext = nc.declare_dram_parameter("wqkv", [D, 3 * QC], BF16, isOutput=False)
    bqk_ext = nc.declare_dram_parameter("bqk", [2 * QC, 1], F32, isOutput=False)
    wpa_ext = nc.declare_dram_parameter("wpa", [QC, D], BF16, isOutput=False)
    out_ext = nc.declare_dram_parameter("out", [SO, D], BF16, isOutput=True)

    partial = nc.dram_tensor("partial", [S, D], BF16)
    rs_out = [
        nc.dram_tensor(f"rs_out{pi}", [nrows // TP, D], BF16)
        for pi, (row0, nrows) in enumerate(RS_PIECES)
    ]

    with TileContext(nc) as tc:
        with (
            tc.tile_pool(name="persist", bufs=1) as persist,
            tc.tile_pool(name="expt_pool", bufs=6) as expt_pool,
            tc.tile_pool(name="mm", bufs=3, space="PSUM") as mm_pool,
            tc.tile_pool(name="pv", bufs=2, space="PSUM") as pv_pool,
            tc.tile_pool(name="small", bufs=4) as small_pool,
            tc.tile_pool(name="ot", bufs=4) as ot_pool,
            tc.tile_pool(name="osb", bufs=6) as osb_pool,
        ):
            # ---- load persistent tiles ----
            # DMA waves ordered by when the compute needs them: biases
            # (tiny), then k-projection w columns + first x chunk (gates
            # the very first scores), then q columns, then v columns
            # (v_tile fillers drain early in stage 0), then the remaining
            # x chunks, then wp (first needed by c_proj in stage 2).
            bqk = []
            for k in range(4):
                t = persist.tile([128, 1], F32, tag=f"bqk{k}", name=f"bqk{k}")
                nc.sync.dma_start(out=t, in_=bqk_ext[k * 128 : (k + 1) * 128, :])
                bqk.append(t)
            xt = []
            wt = []
            for k in range(NK):
                wt.append(
                    persist.tile([128, 3 * QC], BF16, tag=f"wt{k}", name=f"wt{k}")
                )
                xt.append(persist.tile([128, S], BF16, tag=f"xt{k}", name=f"xt{k}"))
            for k in range(NK):
                nc.sync.dma_start(
                    out=wt[k][:, QC : 2 * QC],
                    in_=wqkv_ext[k * 128 : (k + 1) * 128, QC : 2 * QC],
                )
                nc.sync.dma_start(
                    out=xt[k][:, 0:SQB], in_=xT_ext[k * 128 : (k + 1) * 128, 0:SQB]
                )
            for k in range(NK):
                nc.sync.dma_start(
                    out=wt[k][:, 0:QC], in_=wqkv_ext[k * 128 : (k + 1) * 128, 0:QC]
                )
                nc.sync.dma_start(
                    out=xt[k][:, SQB : 2 * SQB],
                    in_=xT_ext[k * 128 : (k + 1) * 128, SQB : 2 * SQB],
                )
            for ns in range(2, NJ):
                for k in range(NK):
                    nc.sync.dma_start(
                        out=xt[k][:, ns * SQB : (ns + 1) * SQB],
                        in_=xT_ext[k * 128 : (k + 1) * 128, ns * SQB : (ns + 1) * SQB],
                    )
            for k in range(NK):
                nc.sync.dma_start(
                    out=wt[k][:, 2 * QC : 3 * QC],
                    in_=wqkv_ext[k * 128 : (k + 1) * 128, 2 * QC : 3 * QC],
                )
            wp = []
            for p in range(2):
                t = persist.tile([128, D], BF16, tag=f"wp{p}", name=f"wp{p}")
                nc.sync.dma_start(out=t, in_=wpa_ext[p * 128 : (p + 1) * 128, :])
                wp.append(t)


            # q/k transposed layout: qk_sb[ct] [128, S], ct 0-1 = q cols,
            # ct 2-3 = k cols; head h lives on partitions (h%2)*64 of
            # tile h//2 (+2 for k).
            qk_sb = [
                persist.tile([128, S], BF16, tag=f"qk{ct}", name=f"qk{ct}")
                for ct in range(4)
            ]

            # v natural layout + ones column: v_sb[t] [128, HPC, 65];
            # [:, h, :64] = v for head h, [:, h, 64] = 1.0
            v_sb = [
                persist.tile([128, HPC, HD + 1], BF16, tag=f"v{t_i}", name=f"v{t_i}")
                for t_i in range(NT)
            ]

            # ---- emission helpers ----
            done = set()  # names of completed queue items

            def qkv_col_tile(ct, ns):
                ps = mm_pool.tile([128, 2, SQB], F32, tag="mm", name="ps_qkv")
                for k in range(NK):
                    nc.tensor.matmul(
                        ps[:, 0, :],
                        wt[k][:, ct * 128 : (ct + 1) * 128],
                        xt[k][:, ns * SQB : (ns + 1) * SQB],
                        start=(k == 0),
                        stop=(k == NK - 1),
                    )
                nc.vector.tensor_scalar_add(
                    qk_sb[ct][:, ns * SQB : (ns + 1) * SQB], ps[:, 0, :], bqk[ct]
                )
                done.add(f"qkv{ct}_{ns}")

            def v_tile(t_i):
                psv = mm_pool.tile([128, 2, HPC, HD], F32, tag="mm", name="ps_v")
                for k in range(NK):
                    nc.tensor.matmul(
                        psv[:, 0, :, :],
                        xt[k][:, t_i * 128 : (t_i + 1) * 128],
                        wt[k][:, 2 * QC : 3 * QC],
                        start=(k == 0),
                        stop=(k == NK - 1),
                    )
                nc.vector.memset(v_sb[t_i][:, :, HD : HD + 1], 1.0)
                nc.vector.tensor_copy(v_sb[t_i][:, :, 0:HD], psv[:, 0, :, :])
                done.add(f"v{t_i}")

            def normalize(h, pv, om):
                rz = small_pool.tile([1, SQB], F32, tag="rz", name="rz")
                nc.vector.reciprocal(rz, pv[HD : HD + 1, :])
                bc = small_pool.tile([HD, SQB], F32, tag="bc", name="bc")
                nc.gpsimd.partition_broadcast(bc, rz)
                if h % 2 == 0:
                    nc.vector.tensor_mul(om[h // 2][0:HD, :], pv[0:HD, :], bc)
                else:
                    o = ot_pool.tile([HD, SQB], BF16, tag="ot", name="ot")
                    nc.vector.tensor_mul(o, pv[0:HD, :], bc)
                    nc.sync.dma_start(out=om[h // 2][HD:128, :], in_=o)

            def cproj_piece(j, om, m, nch):
                pc = mm_pool.tile([128, 2, SQB], F32, tag="mm", name="pc")
                for p in range(2):
                    nc.tensor.matmul(
                        pc[:, 0, :],
                        om[p][:, m * 128 : (m + 1) * 128],
                        wp[p][:, nch * 512 : (nch + 1) * 512],
                        start=(p == 0),
                        stop=(p == 1),
                    )
                osb = osb_pool.tile([128, 512], BF16, tag="osb", name="osb")
                nc.vector.tensor_copy(osb, pc[:, 0, :])
                nc.sync.dma_start(
                    out=partial[
                        j * SQB + m * 128 : j * SQB + (m + 1) * 128,
                        nch * 512 : (nch + 1) * 512,
                    ],
                    in_=osb,
                )

            def rs_piece(pi):
                row0, nrows = RS_PIECES[pi]
                nc.gpsimd.collective_compute(
                    "ReduceScatter",
                    mybir.AluOpType.add,
                    replica_groups=[[0, 1, 2, 3], [4, 5, 6, 7]],
                    ins=[partial[row0 : row0 + nrows, :]],
                    outs=[rs_out[pi].ap()],
                )
                nc.sync.dma_start(
                    out=out_ext[row0 // TP : (row0 + nrows) // TP, :],
                    in_=rs_out[pi][:, :],
                )

            # ---- cycle-budgeted FIFO work queue ----
            queue = deque()
            credit = 0.0

            def push(cost, fn):
                queue.append((cost, fn))

            def drain(budget):
                nonlocal credit
                credit = min(credit + budget, CREDIT_CAP)
                while queue and credit > 0:
                    cost, fn = queue.popleft()
                    fn()
                    credit -= cost

            def drain_until(name):
                # force-drain queue items until `name` has been emitted
                while name not in done:
                    assert queue, f"work queue exhausted waiting for {name}"
                    cost, fn = queue.popleft()
                    fn()

            # ---- prologue: first k group for pair 0 + q chunk 0, then
            # start attention immediately; later k groups, k for pair 1,
            # q(h23, chunk 0) and v tiles drain through the queue (the
            # per-slot guards below force them in time).
            qkv_col_tile(2, 0)
            qkv_col_tile(0, 0)
            for ns in range(1, NJ):
                push(4200, lambda ns=ns: qkv_col_tile(2, ns))
            for ns in range(NJ):
                push(4200, lambda ns=ns: qkv_col_tile(3, ns))
            push(4200, lambda: qkv_col_tile(1, 0))
            for t_i in range(NT):
                push(2300, lambda t_i=t_i: v_tile(t_i))

            om_of = {}

            def pv_unit(j, h, t4, pvp, om):
                # 4 consecutive PV matmuls per unit: fewer PSUM-bank
                # re-entries (each re-entry costs a PE micro-stall).
                for u in range(4):
                    t_i = 4 * t4 + u
                    nc.tensor.matmul(
                        pvp,
                        v_sb[t_i][:, h, :],
                        expt_of[(j, h)][:, t_i, :],
                        start=(t_i == 0),
                        stop=(t_i == NT - 1),
                    )
                if t4 == NT // 4 - 1:
                    normalize(h, pvp, om)

            expt_of = {}

            # ---- paired attention stages ----
            for P in range(2 * NJ):
                j, pr = P // 2, P % 2
                hA, hB = 2 * pr, 2 * pr + 1
                drain_until(f"qkv{pr}_{j}")
                if P == 0:
                    om_of[0] = [
                        ot_pool.tile([128, SQB], BF16, tag="om", name=f"om{p}")
                        for p in range(2)
                    ]
                exptA = expt_pool.tile([128, NT, SQB], BF16, tag="expt", name="exptA")
                exptB = expt_pool.tile([128, NT, SQB], BF16, tag="expt", name="exptB")
                expt_of[(j, hA)] = exptA
                expt_of[(j, hB)] = exptB
                pvpA = pv_pool.tile([HD + 1, SQB], F32, tag="pv", name="pvA")
                pvpB = pv_pool.tile([HD + 1, SQB], F32, tag="pv", name="pvB")
                qsA = qk_sb[pr][0:HD, j * SQB : (j + 1) * SQB]
                qsB = qk_sb[pr][HD:128, j * SQB : (j + 1) * SQB]
                krA = qk_sb[2 + pr][0:HD, :]
                krB = qk_sb[2 + pr][HD:128, :]
                for t2 in range(NT // 2):
                    # drain queued PE work FIRST: the scores matmuls below
                    # may wait on ACT-freed PSUM banks, and the in-order PE
                    # queue would stall ready work stuck behind them.
                    drain(SLOT_BUDGET)
                    # k columns for sk tiles 2*t2, 2*t2+1 must be emitted
                    drain_until(f"qkv{2 + pr}_{(2 * t2 + 1) // 4}")
                    psA = mm_pool.tile([128, 2, SQB], F32, tag="mm", name="ps_sA")
                    psB = mm_pool.tile([128, 2, SQB], F32, tag="mm", name="ps_sB")
                    for u in range(2):
                        t_i = 2 * t2 + u
                        # adjacent row-tiled pair: head A in PE rows 0-63,
                        # head B in rows 64-127 -> concurrent execution
                        nc.tensor.matmul(
                            psA[:, u, :],
                            krA[:, t_i * 128 : (t_i + 1) * 128],
                            qsA,
                            start=True,
                            stop=True,
                        )
                        nc.tensor.matmul(
                            psB[:, u, :],
                            krB[:, t_i * 128 : (t_i + 1) * 128],
                            qsB,
                            start=True,
                            stop=True,
                        )
                    nc.scalar.activation(
                        exptA[:, 2 * t2 : 2 * t2 + 2, :],
                        psA,
                        mybir.ActivationFunctionType.Exp,
                    )
                    nc.scalar.activation(
                        exptB[:, 2 * t2 : 2 * t2 + 2, :],
                        psB,
                        mybir.ActivationFunctionType.Exp,
                    )
                    # rolling-push this pair's PV work (consumed next
                    # stage-ish); one 4-matmul unit per head every 2 slots
                    if t2 % 2 == 1:
                        t4 = t2 // 2
                        push(
                            2200,
                            lambda j=j, hA=hA, t4=t4, pvpA=pvpA, om=om_of[j]: pv_unit(
                                j, hA, t4, pvpA, om
                            ),
                        )
                        push(
                            2200,
                            lambda j=j, hB=hB, t4=t4, pvpB=pvpB, om=om_of[j]: pv_unit(
                                j, hB, t4, pvpB, om
                            ),
                        )
                # end of stage housekeeping
                if pr == 0:
                    # om tiles for next chunk + q tiles for chunk j+1
                    if j + 1 < NJ:
                        om_of[j + 1] = [
                            ot_pool.tile([128, SQB], BF16, tag="om", name=f"om{p}")
                            for p in range(2)
                        ]
                        push(4200, lambda j=j: qkv_col_tile(0, j + 1))
                        push(4200, lambda j=j: qkv_col_tile(1, j + 1))
                else:
                    # c_proj + reduce-scatter for chunk j (after its PVs)
                    def enqueue_cproj(j=j):
                        for m in range(SQB // 128):
                            for nch in range(2):
                                push(
                                    1700,
                                    lambda j=j, m=m, nch=nch: cproj_piece(
                                        j, om_of[j], m, nch
                                    ),
                                )
                            if m == 3:
                                push(0, lambda pi=j: rs_piece(pi))

                    enqueue_cproj()

            # drain everything left: last PVs, last c_proj, last RS
            while queue:
                cost, fn = queue.popleft()
                fn()

    nc.compile()
    return nc


def make_in_maps(x, w_attn, b_attn, w_proj, b_proj):
    bf = ml_dtypes.bfloat16
    in_maps = []
    for c in range(8):
        b = c // TP
        g = c % TP
        cs = slice(g * QC, (g + 1) * QC)
        xT = np.ascontiguousarray(x[b].T).astype(bf)
        wqkv = np.concatenate(
            [w_attn[:, cs], w_attn[:, D:][:, cs], w_attn[:, 2 * D :][:, cs]], axis=1
        ).astype(bf)
        bqk = np.concatenate([b_attn[cs], b_attn[D:][cs]]).reshape(2 * QC, 1)
        bqk = np.ascontiguousarray(bqk, dtype=np.float32)
        wpa = np.ascontiguousarray(w_proj[cs, :].astype(bf))
        in_maps.append({"xT": xT, "wqkv": wqkv, "bqk": bqk, "wpa": wpa})
    return in_maps


def assemble(results):
    # RS piece (row0, nrows) gives core (group rank g) the reduced rows
    # [row0 + g*w, +w) at out rows [row0//TP, +w), w = nrows//TP.
    out = np.empty((B, S, D), np.float32)
    for c in range(8):
        b = c // TP
        g = c % TP
        o = np.asarray(results[c]["out"]).astype(np.float32)
        for row0, nrows in RS_PIECES:
            w = nrows // TP
            out[b, row0 + g * w : row0 + (g + 1) * w, :] = o[
                row0 // TP : row0 // TP + w
            ]
    return out


def kernel(x, w_attn, b_attn, w_proj, b_proj):
    x = np.asarray(x, dtype=np.float32)
    w_attn = np.asarray(w_attn, dtype=np.float32)
    b_attn = np.asarray(b_attn, dtype=np.float32)
    w_proj = np.asarray(w_proj, dtype=np.float32)
    b_proj = np.asarray(b_proj, dtype=np.float32)
    if "nc" not in _CACHE:
        _CACHE["nc"] = build()
    nc = _CACHE["nc"]
    in_maps = make_in_maps(x, w_attn, b_attn, w_proj, b_proj)
    res = run_bass_kernel_spmd(nc, in_maps, core_ids=list(range(8)))
    out = assemble(res.results)
    # biases applied host-side: v-bias shifts attention output by b_v
    # (probs sum to 1), which c_proj maps to b_v @ w_proj; plus b_proj.
    out += b_attn[2 * D :] @ w_proj + b_proj
    return out


# revision 17
# speedup vs baseline: 1.1195x; 1.1195x over previous
"""Distributed attention kernel for 8 TRN2 NeuronCores.

Problem: B=2, S=2048, D=1024, H=16 heads (hd=64), no causal mask, no
scaling.  out = softmax((x@Wq) (x@Wk)^T) (x@Wv) @ Wp + biases.

Sharding: DP=2 over batch x TP=4 over heads.  Core c handles batch c//4
and heads 4*(c%4) .. 4*(c%4)+3, computes its 4 heads' attention plus the
partial c_proj, then a chunked bf16 ReduceScatter(add) over its 4-core
group yields each core's 512-row slice of the final output.

Design notes:
- Scores for a HEAD PAIR are emitted as row-tiled matmul pairs: head A
  occupies PE row strips 0-1 (K rows 0-63), head B strips 2-3 (rows
  64-127); tile_position is inferred from the lhsT/rhs base partitions.
  Adjacent instructions with disjoint row groups execute concurrently in
  the PE array, nearly halving scores time (the K=64 contraction only
  fills half the array otherwise).
- Stages are head-pairs (16 exp instructions each on the ACT engine).
  All other PE work (k/v/q projections for later stages, c_proj of the
  previous chunk, PV matmuls) drains through a cycle-budgeted FIFO work
  queue BEFORE each score slot, so a dependency-stalled scores matmul
  never blocks ready work in the in-order PE queue.
- PV matmuls are rolling-pushed right after their exp instructions in
  4-matmul units (fewer PSUM bank re-entries) and drain ~1 stage later.
- Only the first x column-chunk is DMAed up front so the first QKV
  group starts after ~2.5MB instead of 5.6MB of DMA.
- v-bias and c_proj bias are applied host-side (probs sum to 1, so the
  v-bias shifts attention output by b_v exactly); this removes 48 bias
  matmuls.
- The collective stream runs ReduceScatter pieces serially; the last
  chunk uses a 384+128 row split so the final (exposed) piece is small.

Softmax skips the max-subtraction (scores are O(+-20), exp is safe in
f32): probs = exp(s) / sum exp(s); the denominator comes free as the
65th row of the PV matmul via an appended ones-column on V.

Measured on the 8-core axon trn2 terminal: ~348us (baseline 389us),
rel err 9.56e-3.
"""

import sys
from collections import deque

if "/opt/trn_rl_repo" not in sys.path:
    sys.path.insert(0, "/opt/trn_rl_repo")

import numpy as np
import ml_dtypes

import concourse.bass as bass
import concourse.mybir as mybir
from concourse import bacc
from concourse.tile import TileContext
from concourse.bass_utils import run_bass_kernel_spmd

BF16 = mybir.dt.bfloat16
F32 = mybir.dt.float32

B, S, D = 2, 2048, 1024
H = 16
HD = 64
TP = 4  # tensor-parallel group size (cores per batch)
HPC = H // TP  # heads per core = 4
QC = HPC * HD  # q (or k or v) columns per core = 256
SQB = 512  # sq chunk (free dim of scores/pv matmuls)
NJ = S // SQB  # 4 chunks
NT = S // 128  # 16 sk tiles
NK = D // 128  # 8 contraction tiles for the projections
SO = S // TP  # 512 output rows per core

SLOT_BUDGET = 4200  # queue cycles emitted per score slot
CREDIT_CAP = 8400

# Reduce-scatter pieces (row0, nrows).  Each piece reduces
# partial[row0:row0+nrows] into out_ext[row0//TP:(row0+nrows)//TP]; core
# with group rank g receives reduced rows [row0+g*nrows/TP, +nrows/TP).
# The collective stream executes pieces serially and each piece costs
# ~9.5us fixed + ~18ns/row, so one 512-row piece per chunk is optimal.
RS_PIECES = [(j * 512, 512) for j in range(NJ)]

_CACHE = {}


def build():
    nc = bacc.Bacc(num_devices=8)

    xT_ext = nc.declare_dram_parameter("xT", [D, S], BF16, isOutput=False)
    wqkv_ext = nc.declare_dram_parameter("wqkv", [D, 3 * QC], BF16, isOutput=False)
    bqk_ext = nc.declare_dram_parameter("bqk", [2 * QC, 1], F32, isOutput=False)
    wpa_ext = nc.declare_dram_parameter("wpa", [QC, D], BF16, isOutput=False)
    out_ext = nc.declare_dram_parameter("out", [SO, D], BF16, isOutput=True)

    partial = nc.dram_tensor("partial", [S, D], BF16)
    rs_out = [
        nc.dram_tensor(f"rs_out{pi}", [nrows // TP, D], BF16)
        for pi, (row0, nrows) in enumerate(RS_PIECES)
    ]

    with TileContext(nc) as tc:
        with (
            tc.tile_pool(name="persist", bufs=1) as persist,
            tc.tile_pool(name="expt_pool", bufs=6) as expt_pool,
            tc.tile_pool(name="mm", bufs=3, space="PSUM") as mm_pool,
            tc.tile_pool(name="pv", bufs=2, space="PSUM") as pv_pool,
            tc.tile_pool(name="small", bufs=4) as small_pool,
            tc.tile_pool(name="ot", bufs=4) as ot_pool,
            tc.tile_pool(name="osb", bufs=6) as osb_pool,
        ):
            # ---- load persistent tiles ----
            # DMA waves ordered by when the compute needs them: biases
            # (tiny), then k-projection w columns + first x chunk (gates
            # the very first scores), then q columns, then v columns
            # (v_tile fillers drain early in stage 0), then the remaining
            # x chunks, then wp (first needed by c_proj in stage 2).
            bqk = []
            for k in range(4):
                t = persist.tile([128, 1], F32, tag=f"bqk{k}", name=f"bqk{k}")
                nc.sync.dma_start(out=t, in_=bqk_ext[k * 128 : (k + 1) * 128, :])
                bqk.append(t)
            xt = []
            wt = []
            for k in range(NK):
                wt.append(
                    persist.tile([128, 3 * QC], BF16, tag=f"wt{k}", name=f"wt{k}")
                )
                xt.append(persist.tile([128, S], BF16, tag=f"xt{k}", name=f"xt{k}"))
            for k in range(NK):
                nc.sync.dma_start(
                    out=wt[k][:, QC : 2 * QC],
                    in_=wqkv_ext[k * 128 : (k + 1) * 128, QC : 2 * QC],
                )
                nc.sync.dma_start(
                    out=xt[k][:, 0:SQB], in_=xT_ext[k * 128 : (k + 1) * 128, 0:SQB]
                )
            for k in range(NK):
                nc.sync.dma_start(
                    out=wt[k][:, 0:QC], in_=wqkv_ext[k * 128 : (k + 1) * 128, 0:QC]
                )
                nc.sync.dma_start(
                    out=xt[k][:, SQB : 2 * SQB],
                    in_=xT_ext[k * 128 : (k + 1) * 128, SQB : 2 * SQB],
                )
            for k in range(NK):
                nc.sync.dma_start(
                    out=wt[k][:, 2 * QC : 3 * QC],
                    in_=wqkv_ext[k * 128 : (k + 1) * 128, 2 * QC : 3 * QC],
                )
            for ns in range(2, NJ):
                for k in range(NK):
                    nc.sync.dma_start(
                        out=xt[k][:, ns * SQB : (ns + 1) * SQB],
                        in_=xT_ext[k * 128 : (k + 1) * 128, ns * SQB : (ns + 1) * SQB],
                    )
            wp = []
            for p in range(2):
                t = persist.tile([128, D], BF16, tag=f"wp{p}", name=f"wp{p}")
                nc.sync.dma_start(out=t, in_=wpa_ext[p * 128 : (p + 1) * 128, :])
                wp.append(t)


            # q/k transposed layout: qk_sb[ct] [128, S], ct 0-1 = q cols,
            # ct 2-3 = k cols; head h lives on partitions (h%2)*64 of
            # tile h//2 (+2 for k).
            qk_sb = [
                persist.tile([128, S], BF16, tag=f"qk{ct}", name=f"qk{ct}")
                for ct in range(4)
            ]

            # v natural layout + ones column: v_sb[t] [128, HPC, 65];
            # [:, h, :64] = v for head h, [:, h, 64] = 1.0
            v_sb = [
                persist.tile([128, HPC, HD + 1], BF16, tag=f"v{t_i}", name=f"v{t_i}")
                for t_i in range(NT)
            ]

            # ---- emission helpers ----
            done = set()  # names of completed queue items

            def qkv_col_tile(ct, ns):
                ps = mm_pool.tile([128, 2, SQB], F32, tag="mm", name="ps_qkv")
                for k in range(NK):
                    nc.tensor.matmul(
                        ps[:, 0, :],
                        wt[k][:, ct * 128 : (ct + 1) * 128],
                        xt[k][:, ns * SQB : (ns + 1) * SQB],
                        start=(k == 0),
                        stop=(k == NK - 1),
                    )
                nc.vector.tensor_scalar_add(
                    qk_sb[ct][:, ns * SQB : (ns + 1) * SQB], ps[:, 0, :], bqk[ct]
                )
                done.add(f"qkv{ct}_{ns}")

            def v_tile(t_i):
                psv = mm_pool.tile([128, 2, HPC, HD], F32, tag="mm", name="ps_v")
                for k in range(NK):
                    nc.tensor.matmul(
                        psv[:, 0, :, :],
                        xt[k][:, t_i * 128 : (t_i + 1) * 128],
                        wt[k][:, 2 * QC : 3 * QC],
                        start=(k == 0),
                        stop=(k == NK - 1),
                    )
                nc.vector.memset(v_sb[t_i][:, :, HD : HD + 1], 1.0)
                nc.vector.tensor_copy(v_sb[t_i][:, :, 0:HD], psv[:, 0, :, :])
                done.add(f"v{t_i}")

            def normalize(h, pv, om):
                rz = small_pool.tile([1, SQB], F32, tag="rz", name="rz")
                nc.vector.reciprocal(rz, pv[HD : HD + 1, :])
                bc = small_pool.tile([HD, SQB], F32, tag="bc", name="bc")
                nc.gpsimd.partition_broadcast(bc, rz)
                if h % 2 == 0:
                    nc.vector.tensor_mul(om[h // 2][0:HD, :], pv[0:HD, :], bc)
                else:
                    o = ot_pool.tile([HD, SQB], BF16, tag="ot", name="ot")
                    nc.vector.tensor_mul(o, pv[0:HD, :], bc)
                    nc.sync.dma_start(out=om[h // 2][HD:128, :], in_=o)

            def cproj_piece(j, om, m, nch):
                pc = mm_pool.tile([128, 2, SQB], F32, tag="mm", name="pc")
                for p in range(2):
                    nc.tensor.matmul(
                        pc[:, 0, :],
                        om[p][:, m * 128 : (m + 1) * 128],
                        wp[p][:, nch * 512 : (nch + 1) * 512],
                        start=(p == 0),
                        stop=(p == 1),
                    )
                osb = osb_pool.tile([128, 512], BF16, tag="osb", name="osb")
                nc.vector.tensor_copy(osb, pc[:, 0, :])
                nc.sync.dma_start(
                    out=partial[
                        j * SQB + m * 128 : j * SQB + (m + 1) * 128,
                        nch * 512 : (nch + 1) * 512,
                    ],
                    in_=osb,
                )

            def rs_piece(pi):
                row0, nrows = RS_PIECES[pi]
                nc.gpsimd.collective_compute(
                    "ReduceScatter",
                    mybir.AluOpType.add,
                    replica_groups=[[0, 1, 2, 3], [4, 5, 6, 7]],
                    ins=[partial[row0 : row0 + nrows, :]],
                    outs=[rs_out[pi].ap()],
                )
                nc.sync.dma_start(
                    out=out_ext[row0 // TP : (row0 + nrows) // TP, :],
                    in_=rs_out[pi][:, :],
                )

            # ---- cycle-budgeted FIFO work queue ----
            queue = deque()
            credit = 0.0

            def push(cost, fn):
                queue.append((cost, fn))

            def drain(budget):
                nonlocal credit
                credit = min(credit + budget, CREDIT_CAP)
                while queue and credit > 0:
                    cost, fn = queue.popleft()
                    fn()
                    credit -= cost

            def drain_until(name):
                # force-drain queue items until `name` has been emitted
                while name not in done:
                    assert queue, f"work queue exhausted waiting for {name}"
                    cost, fn = queue.popleft()
                    fn()

            # ---- prologue: first k group for pair 0 + q chunk 0, then
            # start attention immediately; later k groups, k for pair 1,
            # q(h23, chunk 0) and v tiles drain through the queue (the
            # per-slot guards below force them in time).
            qkv_col_tile(2, 0)
            qkv_col_tile(0, 0)
            for ns in range(1, NJ):
                push(4200, lambda ns=ns: qkv_col_tile(2, ns))
            for ns in range(NJ):
                push(4200, lambda ns=ns: qkv_col_tile(3, ns))
            push(4200, lambda: qkv_col_tile(1, 0))
            for t_i in range(NT):
                push(2300, lambda t_i=t_i: v_tile(t_i))

            om_of = {}

            def pv_unit(j, h, t4, pvp, om):
                # 4 consecutive PV matmuls per unit: fewer PSUM-bank
                # re-entries (each re-entry costs a PE micro-stall).
                for u in range(4):
                    t_i = 4 * t4 + u
                    nc.tensor.matmul(
                        pvp,
                        v_sb[t_i][:, h, :],
                        expt_of[(j, h)][:, t_i, :],
                        start=(t_i == 0),
                        stop=(t_i == NT - 1),
                    )
                if t4 == NT // 4 - 1:
                    normalize(h, pvp, om)

            expt_of = {}

            # ---- paired attention stages ----
            for P in range(2 * NJ):
                j, pr = P // 2, P % 2
                hA, hB = 2 * pr, 2 * pr + 1
                drain_until(f"qkv{pr}_{j}")
                if P == 0:
                    om_of[0] = [
                        ot_pool.tile([128, SQB], BF16, tag="om", name=f"om{p}")
                        for p in range(2)
                    ]
                exptA = expt_pool.tile([128, NT, SQB], BF16, tag="expt", name="exptA")
                exptB = expt_pool.tile([128, NT, SQB], BF16, tag="expt", name="exptB")
                expt_of[(j, hA)] = exptA
                expt_of[(j, hB)] = exptB
                pvpA = pv_pool.tile([HD + 1, SQB], F32, tag="pv", name="pvA")
                pvpB = pv_pool.tile([HD + 1, SQB], F32, tag="pv", name="pvB")
                qsA = qk_sb[pr][0:HD, j * SQB : (j + 1) * SQB]
                qsB = qk_sb[pr][HD:128, j * SQB : (j + 1) * SQB]
                krA = qk_sb[2 + pr][0:HD, :]
                krB = qk_sb[2 + pr][HD:128, :]
                for t2 in range(NT // 2):
                    # drain queued PE work FIRST: the scores matmuls below
                    # may wait on ACT-freed PSUM banks, and the in-order PE
                    # queue would stall ready work stuck behind them.
                    drain(SLOT_BUDGET)
                    # k columns for sk tiles 2*t2, 2*t2+1 must be emitted
                    drain_until(f"qkv{2 + pr}_{(2 * t2 + 1) // 4}")
                    psA = mm_pool.tile([128, 2, SQB], F32, tag="mm", name="ps_sA")
                    psB = mm_pool.tile([128, 2, SQB], F32, tag="mm", name="ps_sB")
                    for u in range(2):
                        t_i = 2 * t2 + u
                        # adjacent row-tiled pair: head A in PE rows 0-63,
                        # head B in rows 64-127 -> concurrent execution
                        nc.tensor.matmul(
                            psA[:, u, :],
                            krA[:, t_i * 128 : (t_i + 1) * 128],
                            qsA,
                            start=True,
                            stop=True,
                        )
                        nc.tensor.matmul(
                            psB[:, u, :],
                            krB[:, t_i * 128 : (t_i + 1) * 128],
                            qsB,
                            start=True,
                            stop=True,
                        )
                    nc.scalar.activation(
                        exptA[:, 2 * t2 : 2 * t2 + 2, :],
                        psA,
                        mybir.ActivationFunctionType.Exp,
                    )
                    nc.scalar.activation(
                        exptB[:, 2 * t2 : 2 * t2 + 2, :],
                        psB,
                        mybir.ActivationFunctionType.Exp,
                    )
                    # rolling-push this pair's PV work (consumed next
                    # stage-ish); one 4-matmul unit per head every 2 slots
                    if t2 % 2 == 1:
                        t4 = t2 // 2
                        push(
                            2200,
                            lambda j=j, hA=hA, t4=t4, pvpA=pvpA, om=om_of[j]: pv_unit(
                                j, hA, t4, pvpA, om
                            ),
                        )
                        push(
                            2200,
                            lambda j=j, hB=hB, t4=t4, pvpB=pvpB, om=om_of[j]: pv_unit(
                                j, hB, t4, pvpB, om
                            ),
                        )
                # end of stage housekeeping
                if pr == 0:
                    # om tiles for next chunk + q tiles for chunk j+1
                    if j + 1 < NJ:
                        om_of[j + 1] = [
                            ot_pool.tile([128, SQB], BF16, tag="om", name=f"om{p}")
                            for p in range(2)
                        ]
                        push(4200, lambda j=j: qkv_col_tile(0, j + 1))
                        push(4200, lambda j=j: qkv_col_tile(1, j + 1))
                else:
                    # c_proj + reduce-scatter for chunk j (after its PVs)
                    def enqueue_cproj(j=j):
                        for m in range(SQB // 128):
                            for nch in range(2):
                                push(
                                    1700,
                                    lambda j=j, m=m, nch=nch: cproj_piece(
                                        j, om_of[j], m, nch
                                    ),
                                )
                            if m == 3:
                                push(0, lambda pi=j: rs_piece(pi))

                    enqueue_cproj()

            # drain everything left: last PVs, last c_proj, last RS
            while queue:
                cost, fn = queue.popleft()
                fn()

    nc.compile()
    return nc


def make_in_maps(x, w_attn, b_attn, w_proj, b_proj):
    bf = ml_dtypes.bfloat16
    in_maps = []
    for c in range(8):
        b = c // TP
        g = c % TP
        cs = slice(g * QC, (g + 1) * QC)
        xT = np.ascontiguousarray(x[b].T).astype(bf)
        wqkv = np.concatenate(
            [w_attn[:, cs], w_attn[:, D:][:, cs], w_attn[:, 2 * D :][:, cs]], axis=1
        ).astype(bf)
        bqk = np.concatenate([b_attn[cs], b_attn[D:][cs]]).reshape(2 * QC, 1)
        bqk = np.ascontiguousarray(bqk, dtype=np.float32)
        wpa = np.ascontiguousarray(w_proj[cs, :].astype(bf))
        in_maps.append({"xT": xT, "wqkv": wqkv, "bqk": bqk, "wpa": wpa})
    return in_maps


def assemble(results):
    # RS piece (row0, nrows) gives core (group rank g) the reduced rows
    # [row0 + g*w, +w) at out rows [row0//TP, +w), w = nrows//TP.
    out = np.empty((B, S, D), np.float32)
    for c in range(8):
        b = c // TP
        g = c % TP
        o = np.asarray(results[c]["out"]).astype(np.float32)
        for row0, nrows in RS_PIECES:
            w = nrows // TP
            out[b, row0 + g * w : row0 + (g + 1) * w, :] = o[
                row0 // TP : row0 // TP + w
            ]
    return out


def kernel(x, w_attn, b_attn, w_proj, b_proj):
    x = np.asarray(x, dtype=np.float32)
    w_attn = np.asarray(w_attn, dtype=np.float32)
    b_attn = np.asarray(b_attn, dtype=np.float32)
    w_proj = np.asarray(w_proj, dtype=np.float32)
    b_proj = np.asarray(b_proj, dtype=np.float32)
    if "nc" not in _CACHE:
        _CACHE["nc"] = build()
    nc = _CACHE["nc"]
    in_maps = make_in_maps(x, w_attn, b_attn, w_proj, b_proj)
    res = run_bass_kernel_spmd(nc, in_maps, core_ids=list(range(8)))
    out = assemble(res.results)
    # biases applied host-side: v-bias shifts attention output by b_v
    # (probs sum to 1), which c_proj maps to b_v @ w_proj; plus b_proj.
    out += b_attn[2 * D :] @ w_proj + b_proj
    return out
